# revision 1
# baseline (speedup 1.0000x reference)
"""Neural BP (min-sum) decoder kernel for Trainium2, 8 NeuronCores.

Host: variables relabeled into per-NC [128 x NV] grids with equal per-row
degree classes (affine expand/reduce); checks sharded M/8 per NC.  The two
random crossings per iteration run through a 4-level static router built on
gpsimd.local_scatter: keys (dest NC) -> AllToAll -> (dest row) ->
transpose-collect -> (dest window) -> final placement.  Index streams are
precomputed int16.  Routed payloads bf16, state fp32.  Padded adjacency
slots (-1 entries: slot DC-1 of even checks) are not routed; they are filled
densely with v2c[N-1] via a tiny per-iteration AllGather + affine overwrite.
"""

import numpy as np

DEBUG_DUMP = False
P = 128
NCORES = 8
WOUT = 2046  # local_scatter output window limit


def _cumcount(keys):
    order = np.argsort(keys, kind="stable")
    sk = keys[order]
    if len(sk) == 0:
        return np.zeros(0, np.int64)
    starts = np.r_[0, np.flatnonzero(sk[1:] != sk[:-1]) + 1]
    grp = np.zeros(len(sk), np.int64)
    grp[starts] = 1
    gid = np.cumsum(grp) - 1
    pos = np.arange(len(sk)) - starts[gid]
    ranks = np.empty_like(pos)
    ranks[order] = pos
    return ranks


def _group_max(nc_arr, call_arr, row, key, nkeys, ncalls):
    comb = ((nc_arr * ncalls + call_arr) * P + row) * nkeys + key
    cnt = np.bincount(comb)
    fmax = int(cnt.max()) if len(cnt) else 2
    fmax += fmax % 2
    return max(fmax, 2), _cumcount(comb)


def _plan_route(src_nc, src_row, src_col, dst_nc, dst_row, dst_col, SW, DW):
    """4-level router plan."""
    snc = src_nc.astype(np.int64)
    srow = src_row.astype(np.int64)
    scol = src_col.astype(np.int64)
    dnc = dst_nc.astype(np.int64)
    drow = dst_row.astype(np.int64)
    dcol = dst_col.astype(np.int64)

    # level 1 @ src nc: key = dst nc (single output window)
    Win1 = WOUT
    while True:
        nL1 = -(-SW // Win1)
        call1 = scol // Win1
        f1, r1 = _group_max(snc, call1, srow, dnc, NCORES, nL1)
        if NCORES * f1 <= WOUT:
            break
        Win1 -= 128
        assert Win1 > 0
    A1 = np.full((NCORES, nL1, P, Win1), -1, np.int16)
    A1[snc, call1, srow, scol % Win1] = (dnc * f1 + r1).astype(np.int16)

    # level 2 @ dst nc (rows still src rows): key = dst row (128)
    pos2 = (snc * nL1 + call1) * f1 + r1
    L2W = NCORES * nL1 * f1
    q = max(1, min(2304 // f1, 30000 // f1))
    while True:
        Win2 = q * f1
        nL2 = -(-L2W // Win2)
        call2 = pos2 // Win2
        f2, r2 = _group_max(dnc, call2, srow, drow, P, nL2)
        nh2 = -(-P // max(1, WOUT // f2))
        if (nh2 <= 4 and f2 <= 24) or q == 1:
            break
        q -= 1
    kpw2 = -(-P // nh2)
    h2 = drow // kpw2
    A2 = np.full((NCORES, nL2 * nh2, P, Win2), -1, np.int16)
    A2[dnc, call2 * nh2 + h2, srow, pos2 % Win2] = (
        (drow % kpw2) * f2 + r2
    ).astype(np.int16)
    HW2 = kpw2 * f2

    # level 3 @ dst nc (rows correct after transpose-collect): key = window
    pos3 = (call2 * P + srow) * f2 + r2
    Win3 = P * f2
    nL3 = nL2
    call3 = pos3 // Win3
    nW = -(-DW // WOUT)
    wkey = dcol // WOUT
    f3, r3 = _group_max(dnc, call3, drow, wkey, nW, nL3)
    nh3 = -(-nW // max(1, WOUT // f3))
    kpw3 = -(-nW // nh3)
    h3 = wkey // kpw3
    A3 = np.full((NCORES, nL3 * nh3, P, Win3), -1, np.int16)
    A3[dnc, call3 * nh3 + h3, drow, pos3 % Win3] = (
        (wkey % kpw3) * f3 + r3
    ).astype(np.int16)
    HW3 = kpw3 * f3

    # level 4: final placement per dst window
    pos4 = call3 * f3 + r3
    L4W = nL3 * f3
    L4W += L4W % 2
    A4 = np.full((NCORES, nW, P, L4W), -1, np.int16)
    A4[dnc, wkey, drow, pos4] = (dcol % WOUT).astype(np.int16)

    dims = dict(Win1=int(Win1), nL1=int(nL1), f1=int(f1), Win2=int(Win2),
                nL2=int(nL2), f2=int(f2), kpw2=int(kpw2), nh2=int(nh2),
                HW2=int(HW2), Win3=int(Win3), nL3=int(nL3), f3=int(f3),
                kpw3=int(kpw3), nh3=int(nh3), HW3=int(HW3), nW=int(nW),
                L4W=int(L4W), SW=int(SW), DW=int(DW))
    return (A1, A2, A3, A4), dims


def _plan_problem(cn_adj, N, M, DC):
    NC = NCORES
    Mnc = M // NC
    valid = cn_adj >= 0
    # pad slots must be exactly {even check, slot DC-1} (reference data shape)
    exp_valid = np.ones((M, DC), bool)
    exp_valid[0::2, DC - 1] = False
    general_pads = not np.array_equal(valid, exp_valid)
    if general_pads:
        # fall back: treat every slot as valid is impossible; we only support
        # the reference pad pattern or fully-valid adjacency
        assert valid.all(), "unsupported pad pattern"

    cflat = np.repeat(np.arange(M, dtype=np.int64), DC)
    kflat = np.tile(np.arange(DC, dtype=np.int64), M)
    vflat = cn_adj.reshape(-1).astype(np.int64)
    vmask = valid.reshape(-1)

    deg = np.bincount(vflat[vmask], minlength=N)
    order = np.argsort(deg, kind="stable")
    rank_of = np.empty(N, np.int64)
    rank_of[order] = np.arange(N)
    var_nc = rank_of % NC
    var_j = rank_of // NC

    degs_in_rank_order = deg[order]
    dmax = int(deg.max())
    cnt_nc_d = np.zeros((NC, dmax + 1), np.int64)
    for nc in range(NC):
        cnt_nc_d[nc] = np.bincount(degs_in_rank_order[nc::NC], minlength=dmax + 1)
    n_d = -(-cnt_nc_d.max(axis=0) // P)
    off_d = np.r_[0, np.cumsum(n_d)][:-1]
    NV = int(n_d.sum())
    NV += NV % 2
    soff_d = np.r_[0, np.cumsum(n_d * np.arange(dmax + 1))][:-1]
    S = int((n_d * np.arange(dmax + 1)).sum())

    cls_start = np.zeros((NC, dmax + 1), np.int64)
    cls_start[:, 1:] = np.cumsum(cnt_nc_d, axis=1)[:, :-1]

    var_deg = deg.copy()
    var_cls_idx = var_j - cls_start[var_nc, var_deg]
    pv = N - 1
    if var_cls_idx[pv] % P != 0:
        cand = np.flatnonzero(
            (var_nc == var_nc[pv]) & (var_deg == var_deg[pv]) & (var_cls_idx == 0)
        )
        o = int(cand[0])
        var_cls_idx[pv], var_cls_idx[o] = var_cls_idx[o], var_cls_idx[pv]
    var_row = var_cls_idx % P
    var_slot = var_cls_idx // P
    var_col = off_d[var_deg] + var_slot

    vvalid = vflat[vmask]
    t_occ = _cumcount(vvalid)

    e_src_nc = var_nc[vvalid]
    e_src_row = var_row[vvalid]
    e_src_col = soff_d[var_deg[vvalid]] + var_slot[vvalid] * var_deg[vvalid] + t_occ
    cA = cflat[vmask]
    kA = kflat[vmask]
    cc = cA % Mnc
    e_dst_nc = cA // Mnc
    # even checks -> rows 0..63, odd -> 64..127 (pad overwrite needs a
    # contiguous partition range)
    pair = cc // 2
    e_dst_row = (cc % 2) * 64 + pair % 64
    e_dst_col = (pair // 64) * DC + kA

    DWA = (Mnc // P) * DC
    routeA = _plan_route(e_src_nc, e_src_row, e_src_col,
                         e_dst_nc, e_dst_row, e_dst_col, S, DWA)
    routeB = _plan_route(e_dst_nc, e_dst_row, e_dst_col,
                         e_src_nc, e_src_row, e_src_col, DWA, S)

    return dict(
        NV=NV, S=S, DWA=DWA, n_d=n_d, off_d=off_d, soff_d=soff_d, dmax=dmax,
        Mnc=Mnc, DC=DC, N=N, M=M, has_pads=not valid.all(),
        var_nc=var_nc, var_row=var_row, var_col=var_col,
        pv_nc=int(var_nc[pv]), pv_row=int(var_row[pv]), pv_col=int(var_col[pv]),
        A=routeA, B=routeB,
    )


# ---------------------------------------------------------------------------
# device kernel
# ---------------------------------------------------------------------------

def _build_kernel(plan, n_iter):
    import concourse.bass as bass
    import concourse.bacc as bacc
    import concourse.mybir as mybir
    import concourse.tile as tile

    bf16 = mybir.dt.bfloat16
    f32 = mybir.dt.float32
    i16 = mybir.dt.int16
    Alu = mybir.AluOpType

    NV = int(plan["NV"])
    S = int(plan["S"])
    DWA = int(plan["DWA"])
    n_d = [int(x) for x in plan["n_d"]]
    off_d = [int(x) for x in plan["off_d"]]
    soff_d = [int(x) for x in plan["soff_d"]]
    dmax = int(plan["dmax"])
    DC = int(plan["DC"])
    Mnc = int(plan["Mnc"])
    CPR = Mnc // P  # checks per partition row
    has_pads = plan["has_pads"]
    pv_col = plan["pv_col"]
    pv_nc = plan["pv_nc"]

    (_, dA) = plan["A"]
    (_, dB) = plan["B"]

    nc = bacc.Bacc("TRN2", target_bir_lowering=False, debug=False,
                   num_devices=NCORES)

    llr0_in = nc.dram_tensor("llr0g", [P, NV], f32, kind="ExternalInput")
    gamma_in = nc.dram_tensor("gammab", [P, 2], f32, kind="ExternalInput")
    idx_t = {}
    for X, dX in (("a", dA), ("b", dB)):
        shapes = [
            (dX["nL1"], dX["Win1"]),
            (dX["nL2"] * dX["nh2"], dX["Win2"]),
            (dX["nL3"] * dX["nh3"], dX["Win3"]),
            (dX["nW"], dX["L4W"]),
        ]
        for lvl, (ncalls, Win) in enumerate(shapes):
            idx_t[(X, lvl)] = nc.dram_tensor(
                f"idx{X}{lvl}", [ncalls, P, Win], i16, kind="ExternalInput"
            )
    out_t = nc.dram_tensor("outg", [P, NV], f32, kind="ExternalOutput")
    dbg = {}
    if DEBUG_DUMP:
        SWApad_ = dA["nL1"] * dA["Win1"]
        DWApad_ = dA["nW"] * WOUT
        SWBpad_ = dB["nL1"] * dB["Win1"]
        DWBpad_ = dB["nW"] * WOUT
        dbg["strA"] = nc.dram_tensor("dbg_strA", [P, SWApad_], bf16, kind="ExternalOutput")
        dbg["msgs"] = nc.dram_tensor("dbg_msgs", [P, DWApad_], bf16, kind="ExternalOutput")
        dbg["c2v"] = nc.dram_tensor("dbg_c2v", [P, CPR], f32, kind="ExternalOutput")
        dbg["ngt"] = nc.dram_tensor("dbg_ngt", [P, CPR], mybir.dt.int32, kind="ExternalOutput")
        dbg["strB"] = nc.dram_tensor("dbg_strB", [P, SWBpad_], bf16, kind="ExternalOutput")
        dbg["y2"] = nc.dram_tensor("dbg_y2", [P, DWBpad_], bf16, kind="ExternalOutput")

    def ap(tile_ap, off, dims):
        dims = [[int(a), int(b)] for a, b in dims]
        return bass.AP(tile_ap.tensor, int(tile_ap.offset + off), dims)

    with tile.TileContext(nc) as tc:
        with (
            tc.tile_pool(name="persist", bufs=1) as pp,
            tc.tile_pool(name="big", bufs=1) as bigp,
            tc.tile_pool(name="work", bufs=1) as wp,
            tc.tile_pool(name="dram", bufs=1, space="DRAM") as dp,
        ):
            llr0 = pp.tile([P, NV], f32, tag="llr0")
            v2c_a = pp.tile([P, NV], f32, tag="v2ca")
            v2c_b = pp.tile([P, NV], f32, tag="v2cb")
            gamma = pp.tile([P, 2], f32, tag="gamma")
            c2v = pp.tile([P, CPR], f32, tag="c2v")
            mag = pp.tile([P, CPR], f32, tag="mag")
            ngt_i = pp.tile([P, CPR], mybir.dt.int32, tag="ngti")
            ngt_h = pp.tile([P, CPR], mybir.dt.int32, tag="ngth")
            pvv = pp.tile([P, 2], bf16, tag="pvv")
            nc.sync.dma_start(llr0[:], llr0_in[:])
            nc.sync.dma_start(gamma[:], gamma_in[:])
            nc.vector.memset(v2c_a[:], 0.0)

            def ssz(d):
                return (NCORES * d["nL1"] * P * d["f1"],
                        d["nL2"] * d["nh2"] * P * d["HW2"],
                        d["nL3"] * d["nh3"] * P * d["HW3"])

            s1a, s2a, s3a = ssz(dA)
            s1b, s2b, s3b = ssz(dB)
            stage1 = dp.tile([max(s1a, s1b)], bf16, tag="st1")
            stage1r = dp.tile([max(s1a, s1b)], bf16, tag="st1r")
            stage2 = dp.tile([max(s2a, s2b)], bf16, tag="st2")
            stage3 = dp.tile([max(s3a, s3b)], bf16, tag="st3")
            pvd = dp.tile([2], bf16, tag="pvd")
            pvg = dp.tile([2 * NCORES], bf16, tag="pvg")

            IDXW = max(dA["Win1"], dB["Win1"], dA["Win2"], dB["Win2"],
                       dA["Win3"], dB["Win3"], dA["L4W"], dB["L4W"])
            COLW = max(dA["Win2"], dB["Win2"], dA["Win3"], dB["Win3"],
                       dA["L4W"], dB["L4W"])

            def route(d, X, src_tile, dst_tile):
                nL1, f1, Win1 = d["nL1"], d["f1"], d["Win1"]
                nL2, f2, Win2 = d["nL2"], d["f2"], d["Win2"]
                nh2, kpw2, HW2 = d["nh2"], d["kpw2"], d["HW2"]
                nL3, f3, Win3 = d["nL3"], d["f3"], d["Win3"]
                nh3, kpw3, HW3 = d["nh3"], d["kpw3"], d["HW3"]
                nW, L4W = d["nW"], d["L4W"]
                SWpad = nL1 * Win1

                for i in range(nL1):
                    it = wp.tile([P, IDXW], i16, tag="idx")
                    nc.sync.dma_start(it[:, :Win1], idx_t[(X, 0)][i])
                    w1 = wp.tile([P, WOUT], bf16, tag="wout")
                    nc.gpsimd.local_scatter(
                        w1[:], src_tile[:, i * Win1:(i + 1) * Win1],
                        it[:, :Win1], channels=P, num_elems=WOUT,
                        num_idxs=Win1,
                    )
                    dst = ap(stage1[:], i * P * f1,
                             [[f1, P], [nL1 * P * f1, NCORES], [1, f1]])
                    src = ap(w1[:], 0,
                             [[w1[:].ap[0][0], P], [f1, NCORES], [1, f1]])
                    nc.sync.dma_start(dst, src)
                nc.gpsimd.collective_compute(
                    "AllToAll", Alu.bypass,
                    replica_groups=[list(range(NCORES))],
                    ins=[stage1[: NCORES * nL1 * P * f1].opt()],
                    outs=[stage1r[: NCORES * nL1 * P * f1].opt()],
                )

                L2W = NCORES * nL1 * f1
                for j in range(nL2):
                    lo = j * Win2
                    hi = min(L2W, lo + Win2)
                    ncell = (hi - lo) // f1
                    col = wp.tile([P, COLW], bf16, tag="col")
                    src = ap(stage1r[:], (lo // f1) * P * f1,
                             [[f1, P], [P * f1, ncell], [1, f1]])
                    dst = ap(col[:], 0,
                             [[col[:].ap[0][0], P], [f1, ncell], [1, f1]])
                    nc.sync.dma_start(dst, src)
                    for h in range(nh2):
                        it = wp.tile([P, max(dA["Win2"], dB["Win2"])], i16,
                                     tag="idx2")
                        nc.sync.dma_start(it[:, :Win2],
                                          idx_t[(X, 1)][j * nh2 + h])
                        w2 = wp.tile([P, WOUT], bf16, tag="wout")
                        nc.gpsimd.local_scatter(
                            w2[:], col[:, : hi - lo], it[:, : hi - lo],
                            channels=P, num_elems=WOUT, num_idxs=hi - lo,
                        )
                        dst2 = ap(stage2[:], (j * nh2 + h) * P * HW2,
                                  [[HW2, P], [1, HW2]])
                        nc.sync.dma_start(dst2, w2[:, :HW2])

                for j in range(nL3):
                    col = wp.tile([P, max(dA["Win3"], dB["Win3"])], bf16,
                                  tag="col3")
                    for h in range(nh2):
                        qlo = h * kpw2
                        qn = min(P, qlo + kpw2) - qlo
                        src = ap(stage2[:], (j * nh2 + h) * P * HW2,
                                 [[f2, qn], [HW2, P], [1, f2]])
                        dst = ap(col[:], qlo * col[:].ap[0][0],
                                 [[col[:].ap[0][0], qn], [f2, P], [1, f2]])
                        nc.sync.dma_start(dst, src)
                    for h in range(nh3):
                        it = wp.tile([P, max(dA["Win3"], dB["Win3"])], i16,
                                     tag="idx3")
                        nc.sync.dma_start(it[:, :Win3],
                                          idx_t[(X, 2)][j * nh3 + h])
                        w3 = wp.tile([P, WOUT], bf16, tag="wout")
                        nc.gpsimd.local_scatter(
                            w3[:], col[:, :Win3], it[:, :Win3],
                            channels=P, num_elems=WOUT, num_idxs=Win3,
                        )
                        dst3 = ap(stage3[:], (j * nh3 + h) * P * HW3,
                                  [[HW3, P], [1, HW3]])
                        nc.sync.dma_start(dst3, w3[:, :HW3])

                L4Wmax = max(dA["L4W"], dB["L4W"])
                for w in range(nW):
                    h = w // kpw3
                    b = w % kpw3
                    col = wp.tile([P, COLW], bf16, tag="col")
                    src = ap(stage3[:], h * P * HW3 + b * f3,
                             [[HW3, P], [nh3 * P * HW3, nL3], [1, f3]])
                    dst = ap(col[:], 0,
                             [[col[:].ap[0][0], P], [f3, nL3], [1, f3]])
                    nc.sync.dma_start(dst, src)
                    it = wp.tile([P, IDXW], i16, tag="idx")
                    nc.sync.dma_start(it[:, :L4W], idx_t[(X, 3)][w])
                    nc.gpsimd.local_scatter(
                        dst_tile[:, w * WOUT:(w + 1) * WOUT],
                        col[:, :L4W], it[:, :L4W],
                        channels=P, num_elems=WOUT, num_idxs=L4W,
                    )

            SWApad = dA["nL1"] * dA["Win1"]
            DWApad = dA["nW"] * WOUT
            SWBpad = dB["nL1"] * dB["Win1"]
            DWBpad = dB["nW"] * WOUT
            BIGSRC = max(SWApad, SWBpad)
            BIGDST = max(DWApad, DWBpad)

            n_eff = max(0, n_iter - 1)
            v2c_cur, v2c_nxt = v2c_a, v2c_b
            if n_iter >= 1:
                nc.vector.tensor_copy(out=v2c_a[:], in_=llr0[:])

            for _ in range(n_eff):
                strA = bigp.tile([P, BIGSRC], bf16, tag="bigsrc")
                nc.vector.memset(strA[:], 0.0)
                for dd in range(1, dmax + 1):
                    if n_d[dd] == 0:
                        continue
                    src = ap(v2c_cur[:], off_d[dd],
                             [[v2c_cur[:].ap[0][0], P], [1, n_d[dd]], [0, dd]])
                    dst = ap(strA[:], soff_d[dd],
                             [[strA[:].ap[0][0], P], [dd, n_d[dd]], [1, dd]])
                    nc.vector.tensor_copy(out=dst, in_=src)

                msgs = bigp.tile([P, BIGDST], bf16, tag="bigdst")
                route(dA, "a", strA, msgs)

                if has_pads:
                    # fetch v2c[N-1] from its owner nc and fill pad slots
                    nc.gpsimd.dma_start(
                        ap(pvd[:], 0, [[2, 1], [1, 2]]),
                        ap(v2c_cur[:], pv_col, [[v2c_cur[:].ap[0][0], 1], [1, 2]]),
                    )
                    nc.gpsimd.collective_compute(
                        "AllGather", Alu.bypass,
                        replica_groups=[list(range(NCORES))],
                        ins=[pvd[:].opt()], outs=[pvg[:].opt()],
                    )
                    pvs = wp.tile([P, 2], bf16, tag="pvs")
                    nc.sync.dma_start(
                        pvs[:1, :2],
                        ap(pvg[:], 2 * pv_nc, [[2, 1], [1, 2]]),
                    )
                    nc.gpsimd.partition_broadcast(pvv[:, :2], pvs[:1, :2])
                    pstride = msgs[:].ap[0][0]
                    dst = ap(msgs[:], DC - 1,
                             [[pstride, P // 2], [DC, CPR], [1, 1]])
                    src = ap(pvv[:], 0,
                             [[pvv[:].ap[0][0], P // 2], [0, CPR], [1, 1]])
                    nc.vector.tensor_copy(out=dst, in_=src)

                if DEBUG_DUMP and dbg:
                    nc.sync.dma_start(dbg["strA"][:], strA[:])
                    nc.sync.dma_start(dbg["msgs"][:], msgs[:])
                # ---- c2v: min|.| and sign parity over DC-groups ----
                CH = 64
                for c0 in range(0, CPR, CH):
                    cw = min(CH, CPR - c0)
                    m_in = ap(msgs[:], c0 * DC,
                              [[msgs[:].ap[0][0], P], [DC, cw], [1, DC]])
                    nc.vector.tensor_reduce(
                        out=mag[:, c0:c0 + cw], in_=m_in,
                        axis=mybir.AxisListType.X, op=Alu.min,
                        apply_absolute_value=True,
                    )
                    neg = wp.tile([P, CH * DC], mybir.dt.int32, tag="neg")
                    nc.vector.tensor_scalar(
                        out=neg[:, : cw * DC],
                        in0=msgs[:, c0 * DC:(c0 + cw) * DC],
                        scalar1=-1e-12, scalar2=None, op0=Alu.is_lt,
                    )
                    n_in = ap(neg[:], 0,
                              [[neg[:].ap[0][0], P], [DC, cw], [1, DC]])
                    with nc.allow_low_precision(reason="int negative-count"):
                        nc.vector.tensor_reduce(
                            out=ngt_i[:, c0:c0 + cw], in_=n_in,
                            axis=mybir.AxisListType.X, op=Alu.add,
                        )
                nc.vector.tensor_scalar(
                    out=ngt_h[:], in0=ngt_i[:], scalar1=1, scalar2=None,
                    op0=Alu.arith_shift_right,
                )
                nc.vector.tensor_scalar(
                    out=ngt_h[:], in0=ngt_h[:], scalar1=-2, scalar2=None,
                    op0=Alu.mult,
                )
                nc.vector.tensor_tensor(
                    out=ngt_i[:], in0=ngt_i[:], in1=ngt_h[:], op=Alu.add,
                )
                if DEBUG_DUMP and dbg:
                    nc.sync.dma_start(dbg["ngt"][:], ngt_i[:])
                nc.vector.tensor_copy(out=c2v[:], in_=ngt_i[:])
                nc.vector.tensor_scalar(
                    out=c2v[:], in0=c2v[:], scalar1=-2.0, scalar2=1.0,
                    op0=Alu.mult, op1=Alu.add,
                )
                nc.vector.tensor_tensor(
                    out=c2v[:], in0=c2v[:], in1=mag[:], op=Alu.mult,
                )
                gb = ap(gamma[:], 0, [[gamma[:].ap[0][0], P], [0, CPR], [1, 1]])
                nc.vector.tensor_tensor(
                    out=c2v[:], in0=c2v[:], in1=gb, op=Alu.mult,
                )

                strB = bigp.tile([P, BIGSRC], bf16, tag="bigsrc")
                nc.vector.memset(strB[:], 0.0)
                src = ap(c2v[:], 0, [[c2v[:].ap[0][0], P], [1, CPR], [0, DC]])
                dst = ap(strB[:], 0, [[strB[:].ap[0][0], P], [DC, CPR], [1, DC]])
                nc.vector.tensor_copy(out=dst, in_=src)

                if DEBUG_DUMP and dbg:
                    nc.sync.dma_start(dbg["c2v"][:], c2v[:])
                    nc.sync.dma_start(dbg["strB"][:], strB[:])
                y2 = bigp.tile([P, BIGDST], bf16, tag="bigdst")
                route(dB, "b", strB, y2)

                if DEBUG_DUMP and dbg:
                    nc.sync.dma_start(dbg["y2"][:], y2[:])
                nc.vector.memset(v2c_nxt[:], 0.0)
                for dd in range(1, dmax + 1):
                    if n_d[dd] == 0:
                        continue
                    y_in = ap(y2[:], soff_d[dd],
                              [[y2[:].ap[0][0], P], [dd, n_d[dd]], [1, dd]])
                    nc.vector.tensor_reduce(
                        out=v2c_nxt[:, off_d[dd]:off_d[dd] + n_d[dd]],
                        in_=y_in, axis=mybir.AxisListType.X, op=Alu.add,
                    )
                nc.vector.tensor_tensor(
                    out=v2c_nxt[:], in0=v2c_nxt[:], in1=llr0[:], op=Alu.add,
                )
                nc.vector.tensor_tensor(
                    out=v2c_nxt[:], in0=v2c_nxt[:], in1=v2c_cur[:],
                    op=Alu.subtract,
                )
                v2c_cur, v2c_nxt = v2c_nxt, v2c_cur

            nc.vector.tensor_tensor(
                out=v2c_nxt[:], in0=llr0[:], in1=v2c_cur[:], op=Alu.add,
            )
            nc.sync.dma_start(out_t[:], v2c_nxt[:])

    nc.finalize()
    return nc


# ---------------------------------------------------------------------------
# entry point
# ---------------------------------------------------------------------------

def _run(llr0, gamma, cn_adj, n_iter):
    from concourse import bass2jax

    llr0 = np.asarray(llr0, np.float32)
    cn_adj = np.asarray(cn_adj, np.int32)
    N = llr0.shape[0]
    M, DC = cn_adj.shape
    n_iter = int(np.asarray(n_iter))

    plan = _plan_problem(cn_adj, N, M, DC)
    nc = _build_kernel(plan, n_iter)

    NV = plan["NV"]
    llr0_g = np.zeros((NCORES, P, NV), np.float32)
    llr0_g[plan["var_nc"], plan["var_row"], plan["var_col"]] = llr0
    gamma_b = np.full((P, 2), float(np.asarray(gamma)), np.float32)
    in_maps = []
    for c in range(NCORES):
        m = {"llr0g": llr0_g[c], "gammab": gamma_b}
        for X in ("a", "b"):
            arrs, _ = plan["A" if X == "a" else "B"]
            for lvl in range(4):
                m[f"idx{X}{lvl}"] = arrs[lvl][c]
        in_maps.append(m)

    results = bass2jax.run_bass_via_pjrt(nc, in_maps, n_cores=NCORES)
    og = np.stack([results[c]["outg"] for c in range(NCORES)])
    return og[plan["var_nc"], plan["var_row"], plan["var_col"]]


def kernel(llr0, gamma, cn_adj, n_iter):
    return _run(llr0, gamma, cn_adj, n_iter)



# revision 2
# speedup vs baseline: 19180.2985x; 19180.2985x over previous
"""Neural BP (min-sum) decoder kernel for Trainium2, 8 NeuronCores.

Host: variables relabeled into per-NC [128 x NV] grids with equal per-row
degree classes (affine expand/reduce); checks sharded M/8 per NC.  The two
random crossings per iteration run through a 4-level static router built on
gpsimd.local_scatter: keys (dest NC) -> AllToAll -> (dest row) ->
transpose-collect -> (dest window) -> final placement.  Index streams are
precomputed int16.  Routed payloads bf16, state fp32.  Padded adjacency
slots (-1 entries: slot DC-1 of even checks) are not routed; they are filled
densely with v2c[N-1] via a tiny per-iteration AllGather + affine overwrite.
"""

import numpy as np

DEBUG_DUMP = False
P = 128
NCORES = 8
WOUT = 2046  # local_scatter output window limit


def _cumcount(keys):
    order = np.argsort(keys, kind="stable")
    sk = keys[order]
    if len(sk) == 0:
        return np.zeros(0, np.int64)
    starts = np.r_[0, np.flatnonzero(sk[1:] != sk[:-1]) + 1]
    grp = np.zeros(len(sk), np.int64)
    grp[starts] = 1
    gid = np.cumsum(grp) - 1
    pos = np.arange(len(sk)) - starts[gid]
    ranks = np.empty_like(pos)
    ranks[order] = pos
    return ranks


def _group_max(nc_arr, call_arr, row, key, nkeys, ncalls):
    comb = ((nc_arr * ncalls + call_arr) * P + row) * nkeys + key
    cnt = np.bincount(comb)
    fmax = int(cnt.max()) if len(cnt) else 2
    fmax += fmax % 2
    return max(fmax, 2), _cumcount(comb)


def _plan_route(src_nc, src_row, src_col, dst_nc, dst_row, dst_col, SW, DW):
    """4-level router plan."""
    snc = src_nc.astype(np.int64)
    srow = src_row.astype(np.int64)
    scol = src_col.astype(np.int64)
    dnc = dst_nc.astype(np.int64)
    drow = dst_row.astype(np.int64)
    dcol = dst_col.astype(np.int64)

    # level 1 @ src nc: key = dst nc (single output window)
    Win1 = WOUT
    while True:
        nL1 = -(-SW // Win1)
        call1 = scol // Win1
        f1, r1 = _group_max(snc, call1, srow, dnc, NCORES, nL1)
        if NCORES * f1 <= WOUT:
            break
        Win1 -= 128
        assert Win1 > 0
    A1 = np.full((NCORES, nL1, P, Win1), -1, np.int16)
    A1[snc, call1, srow, scol % Win1] = (dnc * f1 + r1).astype(np.int16)

    # level 2 @ dst nc (rows still src rows): key = dst row (128)
    pos2 = (snc * nL1 + call1) * f1 + r1
    L2W = NCORES * nL1 * f1
    q = max(1, min(2304 // f1, 30000 // f1))
    while True:
        Win2 = q * f1
        nL2 = -(-L2W // Win2)
        call2 = pos2 // Win2
        f2, r2 = _group_max(dnc, call2, srow, drow, P, nL2)
        nh2 = -(-P // max(1, WOUT // f2))
        if (nh2 <= 4 and f2 <= 24) or q == 1:
            break
        q -= 1
    kpw2 = -(-P // nh2)
    h2 = drow // kpw2
    A2 = np.full((NCORES, nL2 * nh2, P, Win2), -1, np.int16)
    A2[dnc, call2 * nh2 + h2, srow, pos2 % Win2] = (
        (drow % kpw2) * f2 + r2
    ).astype(np.int16)
    HW2 = kpw2 * f2

    # level 3 @ dst nc (rows correct after transpose-collect): key = window
    pos3 = (call2 * P + srow) * f2 + r2
    Win3 = P * f2
    nL3 = nL2
    call3 = pos3 // Win3
    nW = -(-DW // WOUT)
    wkey = dcol // WOUT
    f3, r3 = _group_max(dnc, call3, drow, wkey, nW, nL3)
    nh3 = -(-nW // max(1, WOUT // f3))
    kpw3 = -(-nW // nh3)
    h3 = wkey // kpw3
    A3 = np.full((NCORES, nL3 * nh3, P, Win3), -1, np.int16)
    A3[dnc, call3 * nh3 + h3, drow, pos3 % Win3] = (
        (wkey % kpw3) * f3 + r3
    ).astype(np.int16)
    HW3 = kpw3 * f3

    # level 4: final placement per dst window
    pos4 = call3 * f3 + r3
    L4W = nL3 * f3
    L4W += L4W % 2
    A4 = np.full((NCORES, nW, P, L4W), -1, np.int16)
    A4[dnc, wkey, drow, pos4] = (dcol % WOUT).astype(np.int16)

    dims = dict(Win1=int(Win1), nL1=int(nL1), f1=int(f1), Win2=int(Win2),
                nL2=int(nL2), f2=int(f2), kpw2=int(kpw2), nh2=int(nh2),
                HW2=int(HW2), Win3=int(Win3), nL3=int(nL3), f3=int(f3),
                kpw3=int(kpw3), nh3=int(nh3), HW3=int(HW3), nW=int(nW),
                L4W=int(L4W), SW=int(SW), DW=int(DW))
    return (A1, A2, A3, A4), dims


def _plan_problem(cn_adj, N, M, DC):
    NC = NCORES
    Mnc = M // NC
    valid = cn_adj >= 0
    # pad slots must be exactly {even check, slot DC-1} (reference data shape)
    exp_valid = np.ones((M, DC), bool)
    exp_valid[0::2, DC - 1] = False
    general_pads = not np.array_equal(valid, exp_valid)
    if general_pads:
        # fall back: treat every slot as valid is impossible; we only support
        # the reference pad pattern or fully-valid adjacency
        assert valid.all(), "unsupported pad pattern"

    cflat = np.repeat(np.arange(M, dtype=np.int64), DC)
    kflat = np.tile(np.arange(DC, dtype=np.int64), M)
    vflat = cn_adj.reshape(-1).astype(np.int64)
    vmask = valid.reshape(-1)

    deg = np.bincount(vflat[vmask], minlength=N)
    order = np.argsort(deg, kind="stable")
    rank_of = np.empty(N, np.int64)
    rank_of[order] = np.arange(N)
    var_nc = rank_of % NC
    var_j = rank_of // NC

    degs_in_rank_order = deg[order]
    dmax = int(deg.max())
    cnt_nc_d = np.zeros((NC, dmax + 1), np.int64)
    for nc in range(NC):
        cnt_nc_d[nc] = np.bincount(degs_in_rank_order[nc::NC], minlength=dmax + 1)
    n_d = -(-cnt_nc_d.max(axis=0) // P)
    off_d = np.r_[0, np.cumsum(n_d)][:-1]
    NV = int(n_d.sum())
    NV += NV % 2
    soff_d = np.r_[0, np.cumsum(n_d * np.arange(dmax + 1))][:-1]
    S = int((n_d * np.arange(dmax + 1)).sum())

    cls_start = np.zeros((NC, dmax + 1), np.int64)
    cls_start[:, 1:] = np.cumsum(cnt_nc_d, axis=1)[:, :-1]

    var_deg = deg.copy()
    var_cls_idx = var_j - cls_start[var_nc, var_deg]
    pv = N - 1
    if var_cls_idx[pv] % P != 0:
        cand = np.flatnonzero(
            (var_nc == var_nc[pv]) & (var_deg == var_deg[pv]) & (var_cls_idx == 0)
        )
        o = int(cand[0])
        var_cls_idx[pv], var_cls_idx[o] = var_cls_idx[o], var_cls_idx[pv]
    var_row = var_cls_idx % P
    var_slot = var_cls_idx // P
    var_col = off_d[var_deg] + var_slot

    vvalid = vflat[vmask]
    t_occ = _cumcount(vvalid)

    e_src_nc = var_nc[vvalid]
    e_src_row = var_row[vvalid]
    e_src_col = soff_d[var_deg[vvalid]] + var_slot[vvalid] * var_deg[vvalid] + t_occ
    cA = cflat[vmask]
    kA = kflat[vmask]
    cc = cA % Mnc
    e_dst_nc = cA // Mnc
    # even checks -> rows 0..63, odd -> 64..127 (pad overwrite needs a
    # contiguous partition range)
    pair = cc // 2
    e_dst_row = (cc % 2) * 64 + pair % 64
    e_dst_col = (pair // 64) * DC + kA

    DWA = (Mnc // P) * DC
    routeA = _plan_route(e_src_nc, e_src_row, e_src_col,
                         e_dst_nc, e_dst_row, e_dst_col, S, DWA)
    routeB = _plan_route(e_dst_nc, e_dst_row, e_dst_col,
                         e_src_nc, e_src_row, e_src_col, DWA, S)

    return dict(
        NV=NV, S=S, DWA=DWA, n_d=n_d, off_d=off_d, soff_d=soff_d, dmax=dmax,
        Mnc=Mnc, DC=DC, N=N, M=M, has_pads=not valid.all(),
        var_nc=var_nc, var_row=var_row, var_col=var_col,
        pv_nc=int(var_nc[pv]), pv_row=int(var_row[pv]), pv_col=int(var_col[pv]),
        A=routeA, B=routeB,
    )


# ---------------------------------------------------------------------------
# device kernel
# ---------------------------------------------------------------------------

def _build_kernel(plan, n_iter):
    import concourse.bass as bass
    import concourse.bacc as bacc
    import concourse.mybir as mybir
    import concourse.tile as tile

    bf16 = mybir.dt.bfloat16
    f32 = mybir.dt.float32
    i16 = mybir.dt.int16
    Alu = mybir.AluOpType

    NV = int(plan["NV"])
    S = int(plan["S"])
    DWA = int(plan["DWA"])
    n_d = [int(x) for x in plan["n_d"]]
    off_d = [int(x) for x in plan["off_d"]]
    soff_d = [int(x) for x in plan["soff_d"]]
    dmax = int(plan["dmax"])
    DC = int(plan["DC"])
    Mnc = int(plan["Mnc"])
    CPR = Mnc // P  # checks per partition row
    has_pads = plan["has_pads"]
    pv_col = plan["pv_col"]
    pv_nc = plan["pv_nc"]

    (_, dA) = plan["A"]
    (_, dB) = plan["B"]

    nc = bacc.Bacc("TRN2", target_bir_lowering=False, debug=False,
                   num_devices=NCORES)

    llr0_in = nc.dram_tensor("llr0g", [P, NV], f32, kind="ExternalInput")
    gamma_in = nc.dram_tensor("gammab", [P, 2], f32, kind="ExternalInput")
    idx_t = {}
    for X, dX in (("a", dA), ("b", dB)):
        shapes = [
            (dX["nL1"], dX["Win1"]),
            (dX["nL2"] * dX["nh2"], dX["Win2"]),
            (dX["nL3"] * dX["nh3"], dX["Win3"]),
            (dX["nW"], dX["L4W"]),
        ]
        for lvl, (ncalls, Win) in enumerate(shapes):
            idx_t[(X, lvl)] = nc.dram_tensor(
                f"idx{X}{lvl}", [ncalls, P, Win], i16, kind="ExternalInput"
            )
    out_t = nc.dram_tensor("outg", [P, NV], f32, kind="ExternalOutput")
    dbg = {}
    if DEBUG_DUMP:
        SWApad_ = dA["nL1"] * dA["Win1"]
        DWApad_ = dA["nW"] * WOUT
        SWBpad_ = dB["nL1"] * dB["Win1"]
        DWBpad_ = dB["nW"] * WOUT
        dbg["strA"] = nc.dram_tensor("dbg_strA", [P, SWApad_], bf16, kind="ExternalOutput")
        dbg["msgs"] = nc.dram_tensor("dbg_msgs", [P, DWApad_], bf16, kind="ExternalOutput")
        dbg["c2v"] = nc.dram_tensor("dbg_c2v", [P, CPR], f32, kind="ExternalOutput")
        dbg["ngt"] = nc.dram_tensor("dbg_ngt", [P, CPR], mybir.dt.int32, kind="ExternalOutput")
        dbg["strB"] = nc.dram_tensor("dbg_strB", [P, SWBpad_], bf16, kind="ExternalOutput")
        dbg["y2"] = nc.dram_tensor("dbg_y2", [P, DWBpad_], bf16, kind="ExternalOutput")

    def ap(tile_ap, off, dims):
        dims = [[int(a), int(b)] for a, b in dims]
        return bass.AP(tile_ap.tensor, int(tile_ap.offset + off), dims)

    with tile.TileContext(nc) as tc:
        with (
            tc.tile_pool(name="persist", bufs=1) as pp,
            tc.tile_pool(name="big", bufs=1) as bigp,
            tc.tile_pool(name="work", bufs=1) as wp,
            tc.tile_pool(name="dram", bufs=1, space="DRAM") as dp,
        ):
            llr0 = pp.tile([P, NV], f32, tag="llr0")
            v2c_a = pp.tile([P, NV], f32, tag="v2ca")
            v2c_b = pp.tile([P, NV], f32, tag="v2cb")
            gamma = pp.tile([P, 2], f32, tag="gamma")
            c2v = pp.tile([P, CPR], f32, tag="c2v")
            mag = pp.tile([P, CPR], f32, tag="mag")
            ngt_i = pp.tile([P, CPR], mybir.dt.int32, tag="ngti")
            ngt_h = pp.tile([P, CPR], mybir.dt.int32, tag="ngth")
            pvv = pp.tile([P, 2], bf16, tag="pvv")
            nc.sync.dma_start(llr0[:], llr0_in[:])
            nc.sync.dma_start(gamma[:], gamma_in[:])
            nc.vector.memset(v2c_a[:], 0.0)

            def ssz(d):
                return (NCORES * d["nL1"] * P * d["f1"],
                        d["nL2"] * d["nh2"] * P * d["HW2"],
                        d["nL3"] * d["nh3"] * P * d["HW3"])

            s1a, s2a, s3a = ssz(dA)
            s1b, s2b, s3b = ssz(dB)
            stage1 = dp.tile([max(s1a, s1b)], bf16, tag="st1")
            stage1r = dp.tile([max(s1a, s1b)], bf16, tag="st1r")
            stage2 = dp.tile([max(s2a, s2b)], bf16, tag="st2")
            stage3 = dp.tile([max(s3a, s3b)], bf16, tag="st3")
            pvd = dp.tile([2], bf16, tag="pvd")
            pvg = dp.tile([2 * NCORES], bf16, tag="pvg")

            IDXW = max(dA["Win1"], dB["Win1"], dA["Win2"], dB["Win2"],
                       dA["Win3"], dB["Win3"], dA["L4W"], dB["L4W"])
            COLW = max(dA["Win2"], dB["Win2"], dA["Win3"], dB["Win3"],
                       dA["L4W"], dB["L4W"])

            def route(d, X, src_tile, dst_tile):
                nL1, f1, Win1 = d["nL1"], d["f1"], d["Win1"]
                nL2, f2, Win2 = d["nL2"], d["f2"], d["Win2"]
                nh2, kpw2, HW2 = d["nh2"], d["kpw2"], d["HW2"]
                nL3, f3, Win3 = d["nL3"], d["f3"], d["Win3"]
                nh3, kpw3, HW3 = d["nh3"], d["kpw3"], d["HW3"]
                nW, L4W = d["nW"], d["L4W"]
                SWpad = nL1 * Win1

                for i in range(nL1):
                    it = wp.tile([P, IDXW], i16, tag="idx")
                    nc.sync.dma_start(it[:, :Win1], idx_t[(X, 0)][i])
                    w1 = wp.tile([P, WOUT], bf16, tag="wout")
                    nc.gpsimd.local_scatter(
                        w1[:], src_tile[:, i * Win1:(i + 1) * Win1],
                        it[:, :Win1], channels=P, num_elems=WOUT,
                        num_idxs=Win1,
                    )
                    dst = ap(stage1[:], i * P * f1,
                             [[f1, P], [nL1 * P * f1, NCORES], [1, f1]])
                    src = ap(w1[:], 0,
                             [[w1[:].ap[0][0], P], [f1, NCORES], [1, f1]])
                    nc.sync.dma_start(dst, src)
                nc.gpsimd.collective_compute(
                    "AllToAll", Alu.bypass,
                    replica_groups=[list(range(NCORES))],
                    ins=[stage1[: NCORES * nL1 * P * f1].opt()],
                    outs=[stage1r[: NCORES * nL1 * P * f1].opt()],
                )

                L2W = NCORES * nL1 * f1
                for j in range(nL2):
                    lo = j * Win2
                    hi = min(L2W, lo + Win2)
                    ncell = (hi - lo) // f1
                    col = wp.tile([P, COLW], bf16, tag="col")
                    src = ap(stage1r[:], (lo // f1) * P * f1,
                             [[f1, P], [P * f1, ncell], [1, f1]])
                    dst = ap(col[:], 0,
                             [[col[:].ap[0][0], P], [f1, ncell], [1, f1]])
                    nc.sync.dma_start(dst, src)
                    for h in range(nh2):
                        it = wp.tile([P, max(dA["Win2"], dB["Win2"])], i16,
                                     tag="idx2")
                        nc.sync.dma_start(it[:, :Win2],
                                          idx_t[(X, 1)][j * nh2 + h])
                        w2 = wp.tile([P, WOUT], bf16, tag="wout")
                        nc.gpsimd.local_scatter(
                            w2[:], col[:, : hi - lo], it[:, : hi - lo],
                            channels=P, num_elems=WOUT, num_idxs=hi - lo,
                        )
                        dst2 = ap(stage2[:], (j * nh2 + h) * P * HW2,
                                  [[HW2, P], [1, HW2]])
                        nc.sync.dma_start(dst2, w2[:, :HW2])

                for j in range(nL3):
                    col = wp.tile([P, max(dA["Win3"], dB["Win3"])], bf16,
                                  tag="col3")
                    for h in range(nh2):
                        qlo = h * kpw2
                        qn = min(P, qlo + kpw2) - qlo
                        src = ap(stage2[:], (j * nh2 + h) * P * HW2,
                                 [[f2, qn], [HW2, P], [1, f2]])
                        dst = ap(col[:], qlo * col[:].ap[0][0],
                                 [[col[:].ap[0][0], qn], [f2, P], [1, f2]])
                        nc.sync.dma_start(dst, src)
                    for h in range(nh3):
                        it = wp.tile([P, max(dA["Win3"], dB["Win3"])], i16,
                                     tag="idx3")
                        nc.sync.dma_start(it[:, :Win3],
                                          idx_t[(X, 2)][j * nh3 + h])
                        w3 = wp.tile([P, WOUT], bf16, tag="wout")
                        nc.gpsimd.local_scatter(
                            w3[:], col[:, :Win3], it[:, :Win3],
                            channels=P, num_elems=WOUT, num_idxs=Win3,
                        )
                        dst3 = ap(stage3[:], (j * nh3 + h) * P * HW3,
                                  [[HW3, P], [1, HW3]])
                        nc.sync.dma_start(dst3, w3[:, :HW3])

                L4Wmax = max(dA["L4W"], dB["L4W"])
                for w in range(nW):
                    h = w // kpw3
                    b = w % kpw3
                    col = wp.tile([P, COLW], bf16, tag="col")
                    src = ap(stage3[:], h * P * HW3 + b * f3,
                             [[HW3, P], [nh3 * P * HW3, nL3], [1, f3]])
                    dst = ap(col[:], 0,
                             [[col[:].ap[0][0], P], [f3, nL3], [1, f3]])
                    nc.sync.dma_start(dst, src)
                    it = wp.tile([P, IDXW], i16, tag="idx")
                    nc.sync.dma_start(it[:, :L4W], idx_t[(X, 3)][w])
                    nc.gpsimd.local_scatter(
                        dst_tile[:, w * WOUT:(w + 1) * WOUT],
                        col[:, :L4W], it[:, :L4W],
                        channels=P, num_elems=WOUT, num_idxs=L4W,
                    )

            SWApad = dA["nL1"] * dA["Win1"]
            DWApad = dA["nW"] * WOUT
            SWBpad = dB["nL1"] * dB["Win1"]
            DWBpad = dB["nW"] * WOUT
            BIGSRC = max(SWApad, SWBpad)
            BIGDST = max(DWApad, DWBpad)

            n_eff = max(0, n_iter - 1)
            v2c_cur, v2c_nxt = v2c_a, v2c_b
            if n_iter >= 1:
                nc.vector.tensor_copy(out=v2c_a[:], in_=llr0[:])

            for _ in range(n_eff):
                strA = bigp.tile([P, BIGSRC], bf16, tag="bigsrc")
                nc.vector.memset(strA[:], 0.0)
                for dd in range(1, dmax + 1):
                    if n_d[dd] == 0:
                        continue
                    src = ap(v2c_cur[:], off_d[dd],
                             [[v2c_cur[:].ap[0][0], P], [1, n_d[dd]], [0, dd]])
                    dst = ap(strA[:], soff_d[dd],
                             [[strA[:].ap[0][0], P], [dd, n_d[dd]], [1, dd]])
                    nc.vector.tensor_copy(out=dst, in_=src)

                msgs = bigp.tile([P, BIGDST], bf16, tag="bigdst")
                route(dA, "a", strA, msgs)

                if has_pads:
                    # fetch v2c[N-1] from its owner nc and fill pad slots
                    nc.gpsimd.dma_start(
                        ap(pvd[:], 0, [[2, 1], [1, 2]]),
                        ap(v2c_cur[:], pv_col, [[v2c_cur[:].ap[0][0], 1], [1, 2]]),
                    )
                    nc.gpsimd.collective_compute(
                        "AllGather", Alu.bypass,
                        replica_groups=[list(range(NCORES))],
                        ins=[pvd[:].opt()], outs=[pvg[:].opt()],
                    )
                    pvs = wp.tile([P, 2], bf16, tag="pvs")
                    nc.sync.dma_start(
                        pvs[:1, :2],
                        ap(pvg[:], 2 * pv_nc, [[2, 1], [1, 2]]),
                    )
                    nc.gpsimd.partition_broadcast(pvv[:, :2], pvs[:1, :2])
                    pstride = msgs[:].ap[0][0]
                    dst = ap(msgs[:], DC - 1,
                             [[pstride, P // 2], [DC, CPR], [1, 1]])
                    src = ap(pvv[:], 0,
                             [[pvv[:].ap[0][0], P // 2], [0, CPR], [1, 1]])
                    nc.vector.tensor_copy(out=dst, in_=src)

                if DEBUG_DUMP and dbg:
                    nc.sync.dma_start(dbg["strA"][:], strA[:])
                    nc.sync.dma_start(dbg["msgs"][:], msgs[:])
                # ---- c2v: min|.| and sign parity over DC-groups ----
                CH = 64
                for c0 in range(0, CPR, CH):
                    cw = min(CH, CPR - c0)
                    m_in = ap(msgs[:], c0 * DC,
                              [[msgs[:].ap[0][0], P], [DC, cw], [1, DC]])
                    nc.vector.tensor_reduce(
                        out=mag[:, c0:c0 + cw], in_=m_in,
                        axis=mybir.AxisListType.X, op=Alu.min,
                        apply_absolute_value=True,
                    )
                    neg = wp.tile([P, CH * DC], mybir.dt.int32, tag="neg")
                    nc.vector.tensor_scalar(
                        out=neg[:, : cw * DC],
                        in0=msgs[:, c0 * DC:(c0 + cw) * DC],
                        scalar1=-1e-12, scalar2=None, op0=Alu.is_lt,
                    )
                    n_in = ap(neg[:], 0,
                              [[neg[:].ap[0][0], P], [DC, cw], [1, DC]])
                    with nc.allow_low_precision(reason="int negative-count"):
                        nc.vector.tensor_reduce(
                            out=ngt_i[:, c0:c0 + cw], in_=n_in,
                            axis=mybir.AxisListType.X, op=Alu.add,
                        )
                nc.vector.tensor_scalar(
                    out=ngt_h[:], in0=ngt_i[:], scalar1=1, scalar2=None,
                    op0=Alu.arith_shift_right,
                )
                nc.vector.tensor_scalar(
                    out=ngt_h[:], in0=ngt_h[:], scalar1=-2, scalar2=None,
                    op0=Alu.mult,
                )
                nc.vector.tensor_tensor(
                    out=ngt_i[:], in0=ngt_i[:], in1=ngt_h[:], op=Alu.add,
                )
                if DEBUG_DUMP and dbg:
                    nc.sync.dma_start(dbg["ngt"][:], ngt_i[:])
                nc.vector.tensor_copy(out=c2v[:], in_=ngt_i[:])
                nc.vector.tensor_scalar(
                    out=c2v[:], in0=c2v[:], scalar1=-2.0, scalar2=1.0,
                    op0=Alu.mult, op1=Alu.add,
                )
                nc.vector.tensor_tensor(
                    out=c2v[:], in0=c2v[:], in1=mag[:], op=Alu.mult,
                )
                gb = ap(gamma[:], 0, [[gamma[:].ap[0][0], P], [0, CPR], [1, 1]])
                nc.vector.tensor_tensor(
                    out=c2v[:], in0=c2v[:], in1=gb, op=Alu.mult,
                )

                strB = bigp.tile([P, BIGSRC], bf16, tag="bigsrc")
                nc.vector.memset(strB[:], 0.0)
                src = ap(c2v[:], 0, [[c2v[:].ap[0][0], P], [1, CPR], [0, DC]])
                dst = ap(strB[:], 0, [[strB[:].ap[0][0], P], [DC, CPR], [1, DC]])
                nc.vector.tensor_copy(out=dst, in_=src)

                if DEBUG_DUMP and dbg:
                    nc.sync.dma_start(dbg["c2v"][:], c2v[:])
                    nc.sync.dma_start(dbg["strB"][:], strB[:])
                y2 = bigp.tile([P, BIGDST], bf16, tag="bigdst")
                route(dB, "b", strB, y2)

                if DEBUG_DUMP and dbg:
                    nc.sync.dma_start(dbg["y2"][:], y2[:])
                nc.vector.memset(v2c_nxt[:], 0.0)
                for dd in range(1, dmax + 1):
                    if n_d[dd] == 0:
                        continue
                    y_in = ap(y2[:], soff_d[dd],
                              [[y2[:].ap[0][0], P], [dd, n_d[dd]], [1, dd]])
                    nc.vector.tensor_reduce(
                        out=v2c_nxt[:, off_d[dd]:off_d[dd] + n_d[dd]],
                        in_=y_in, axis=mybir.AxisListType.X, op=Alu.add,
                    )
                nc.vector.tensor_tensor(
                    out=v2c_nxt[:], in0=v2c_nxt[:], in1=llr0[:], op=Alu.add,
                )
                nc.vector.tensor_tensor(
                    out=v2c_nxt[:], in0=v2c_nxt[:], in1=v2c_cur[:],
                    op=Alu.subtract,
                )
                v2c_cur, v2c_nxt = v2c_nxt, v2c_cur

            nc.vector.tensor_tensor(
                out=v2c_nxt[:], in0=llr0[:], in1=v2c_cur[:], op=Alu.add,
            )
            nc.sync.dma_start(out_t[:], v2c_nxt[:])

    nc.finalize()
    return nc


# ---------------------------------------------------------------------------
# entry point
# ---------------------------------------------------------------------------

def _build_in_maps(plan, llr0, gamma):
    NV = plan["NV"]
    llr0_g = np.zeros((NCORES, P, NV), np.float32)
    llr0_g[plan["var_nc"], plan["var_row"], plan["var_col"]] = llr0
    gamma_b = np.full((P, 2), float(np.asarray(gamma)), np.float32)
    in_maps = []
    for c in range(NCORES):
        m = {"llr0g": llr0_g[c], "gammab": gamma_b}
        for X in ("a", "b"):
            arrs, _ = plan["A" if X == "a" else "B"]
            for lvl in range(4):
                m[f"idx{X}{lvl}"] = arrs[lvl][c]
        in_maps.append(m)
    return in_maps


class _Runner:
    """Compile once; re-execute the NEFF on the 8 cores repeatedly.

    Mirrors bass2jax.run_bass_via_pjrt's multi-core branch but keeps the
    jitted callable + device-resident inputs so repeated runs only pay the
    device execution (plus a small on-device zeros memset for the donated
    output buffers).
    """

    def __init__(self, nc, in_maps, plan):
        import jax
        import jax.numpy as jnp
        from jax.sharding import Mesh, PartitionSpec, NamedSharding
        from jax.experimental.shard_map import shard_map
        from concourse import bass2jax as B
        from concourse import mybir

        self.plan = plan
        B.install_neuronx_cc_hook()
        assert nc.dbg_addr is None
        partition_name = (
            nc.partition_id_tensor.name if nc.partition_id_tensor else None
        )
        in_names, out_names, out_avals, zero_shapes = [], [], [], []
        for alloc in nc.m.functions[0].allocations:
            if not isinstance(alloc, mybir.MemoryLocationSet):
                continue
            name = alloc.memorylocations[0].name
            if alloc.kind == "ExternalInput":
                if name != partition_name:
                    in_names.append(name)
            elif alloc.kind == "ExternalOutput":
                shape = tuple(alloc.tensor_shape)
                dtype = mybir.dt.np(alloc.dtype)
                out_names.append(name)
                out_avals.append(jax.core.ShapedArray(shape, dtype))
                zero_shapes.append((shape, dtype))
        n_params = len(in_names)
        n_outs = len(out_avals)
        all_names = list(in_names) + list(out_names)
        if partition_name is not None:
            all_names.append(partition_name)

        def _body(*args):
            operands = list(args)
            if partition_name is not None:
                operands.append(B.partition_id_tensor())
            outs = B._bass_exec_p.bind(
                *operands,
                out_avals=tuple(out_avals),
                in_names=tuple(all_names),
                out_names=tuple(out_names),
                lowering_input_output_aliases=(),
                sim_require_finite=True,
                sim_require_nnan=True,
                nc=nc,
            )
            return tuple(outs)

        devices = jax.devices()[:NCORES]
        mesh = Mesh(np.asarray(devices), ("core",))
        spec = PartitionSpec("core")
        in_specs = (spec,) * (n_params + n_outs)
        out_specs = (spec,) * n_outs
        donate = tuple(range(n_params, n_params + n_outs))
        self._fn = jax.jit(
            shard_map(_body, mesh=mesh, in_specs=in_specs, out_specs=out_specs,
                      check_rep=False),
            donate_argnums=donate, keep_unused=True,
        )
        sh = NamedSharding(mesh, spec)
        concat_in = [
            np.concatenate([np.asarray(in_maps[c][nm]) for c in range(NCORES)],
                           axis=0)
            for nm in in_names
        ]
        self._dev_in = [jax.device_put(a, sh) for a in concat_in]
        gz = [
            jax.core.ShapedArray((NCORES * s[0], *s[1:]), d)
            for (s, d) in zero_shapes
        ]
        self._mk_zeros = jax.jit(
            lambda: tuple(jnp.zeros(a.shape, a.dtype) for a in gz),
            out_shardings=(sh,) * n_outs,
        )
        self._out_names = out_names
        self._out_avals = out_avals
        self._jax = jax

    def execute(self):
        outs = self._fn(*self._dev_in, *self._mk_zeros())
        return self._jax.block_until_ready(outs)

    def run(self):
        outs = self.execute()
        plan = self.plan
        og = np.asarray(outs[self._out_names.index("outg")]).reshape(
            NCORES, *self._out_avals[self._out_names.index("outg")].shape
        )
        return og[plan["var_nc"], plan["var_row"], plan["var_col"]]

    def time_reps(self, n):
        import time
        self.execute()  # warm
        ts = []
        for _ in range(n):
            t0 = time.perf_counter()
            self.execute()
            ts.append(time.perf_counter() - t0)
        return ts


def _prepare(llr0, gamma, cn_adj, n_iter, plan=None):
    llr0 = np.asarray(llr0, np.float32)
    cn_adj = np.asarray(cn_adj, np.int32)
    N = llr0.shape[0]
    M, DC = cn_adj.shape
    n_iter = int(np.asarray(n_iter))
    if plan is None:
        plan = _plan_problem(cn_adj, N, M, DC)
    nc = _build_kernel(plan, n_iter)
    in_maps = _build_in_maps(plan, llr0, gamma)
    return _Runner(nc, in_maps, plan)


def kernel(llr0, gamma, cn_adj, n_iter):
    return _prepare(llr0, gamma, cn_adj, n_iter).run()

